# revision 15
# baseline (speedup 1.0000x reference)
"""Trainium2 Bass kernel for nn_DCP_LOSS (stain-deconvolution loss).

Data-parallel over batch: B=8 -> 8 NeuronCores, one batch item per core.
Per core, for its (input, target) pair and both stains (h, d):
  ln(clamped rgb) -> channel mix via PE diag-matmuls -> 3x exp (coeffs folded
  into exp bias) -> PE identity-accumulate (f32r) -> ln(G+calib) -> fod.
  Outputs: masks (fod >= 0.3), block sums of fod_relu, and a 20-bin weighted
  histogram recovered from M_k = sum fod*(fod >= t_k) via
    value family  V_k = sum max(fod, t_k)        (DVE, f32 2x)  == r_k + t_k*N
              or  r_k = sum relu(fod - t_k)      (ACT, accum)
    count family  C_k = sum (code >= k)          (DVE, bf16 4x on int codes)
    M_k = r_k + t_k*C_k ;  bin_k = M_k - M_{k+1}
Host combines the tiny per-core stats into the final scalar loss.

The math restructuring (verified exact): all reference clips only matter
where fod == 0, which contributes 0 to every reduction, so
fod = max(-log10(sum_d exp(M[row,d]*s' + ln c_d) + calib), 0) with
s' = sum_c HED[c,col]*ln(max(rgb_c, 1e-6)).
"""

import math
import os
import sys

sys.path.insert(0, "/opt/trn_rl_repo")

import numpy as np


def _setup_act_tables():
    """Point walrus at an act_info.json whose first set holds BOTH ln and
    exp, so the per-band ln/exp interleave does not thrash ACT_TABLE_LOAD."""
    if not os.environ.get("DCP_ACT_TABLE_FIX"):  # crashes remote NRT; keep off
        return
    if os.environ.get("BASS_ACT_ROOT_JSON_PATH"):
        return
    import glob
    import json

    cands = glob.glob(
        "/nix/store/*/lib/python3.13/site-packages/neuronxcc/pwp/"
        "pwp_bin_trainium/act_info.json"
    )
    if not cands:
        return
    src = cands[0]
    srcdir = os.path.dirname(src)
    dst = "/tmp/dcp_pwp"
    os.makedirs(dst, exist_ok=True)
    for f in os.listdir(srcdir):
        t = os.path.join(dst, f)
        if not os.path.exists(t):
            try:
                os.symlink(os.path.join(srcdir, f), t)
            except OSError:
                pass
    d = json.load(open(src))
    sets = d["act_func_sets"]
    first = [s for s in sets if s["name"] == "natural_log_exp_and_others"]
    rest = [s for s in sets if s["name"] != "natural_log_exp_and_others"]
    d["act_func_sets"] = first + rest
    out = os.path.join(dst, "act_info.json")
    os.unlink(out) if os.path.islink(out) else None
    with open(out, "w") as f:
        json.dump(d, f)
    os.environ["BASS_ACT_ROOT_JSON_PATH"] = out


_setup_act_tables()

import concourse.bacc as bacc
import concourse.bass as bass
import concourse.mybir as mybir
import concourse.tile as tile
from concourse import bass_utils

f32 = mybir.dt.float32
f32r = mybir.dt.float32r
bf16 = mybir.dt.bfloat16
i32 = mybir.dt.int32

# ---- constants (from the reference) ----
_RGB_FROM_HED = np.array(
    [[0.65, 0.7, 0.29], [0.07, 0.99, 0.11], [0.27, 0.57, 0.78]], dtype=np.float64
)
_HED_FROM_RGB = np.linalg.inv(_RGB_FROM_HED)
_COEFFS = np.array([0.2125, 0.7154, 0.0721], dtype=np.float64)
_CAL = 10.0 ** (-math.e)
_T_FOD = 0.15
_T_MASK = 0.3
_NUM_BINS = 20
_BW32 = np.float32(math.e) / np.float32(20.0)
_INV_LN10 = 1.0 / math.log(10.0)

B, C, H, W = 8, 3, 1024, 1024
P = 128

# bins whose value-measurement runs on ACT (r_k = sum relu(fod-t_k));
# the rest run on DVE (V_k = sum max(fod, t_k)).
ACT_BINS = (() if os.environ.get("DCP_NO_ACT_ACCUM")
            else tuple(range(10, 20)))
USE_F32R = not os.environ.get("DCP_NO_F32R")
WG = 4  # bands per wide histogram group


def _diag_consts() -> np.ndarray:
    out = np.zeros((8, P, P), dtype=np.float32)
    eye = np.eye(P, dtype=np.float32)
    for c in range(3):
        out[c] = np.float32(_HED_FROM_RGB[c, 0]) * eye
        out[3 + c] = np.float32(_HED_FROM_RGB[c, 2]) * eye
    out[6] = eye
    out[7] = eye  # f32r identity (same bits)
    return out


def build_program(nc, Himg: int, Wimg: int):
    nbands = Himg // P
    mmchunk = min(512, Wimg)
    wg = min(WG, nbands)
    ng = nbands // wg
    assert Himg % P == 0 and Wimg % mmchunk == 0 and nbands % wg == 0
    wide = wg * Wimg

    x_d = nc.dram_tensor("x", (3, Himg, Wimg), f32, kind="ExternalInput")
    y_d = nc.dram_tensor("y", (3, Himg, Wimg), f32, kind="ExternalInput")
    diag_d = nc.dram_tensor("diags", (7, P, P), f32, kind="ExternalInput")
    identr_d = nc.dram_tensor("identr", (P, P), f32r, kind="ExternalInput")

    masks_d = [
        nc.dram_tensor(n, (Himg, Wimg), f32, kind="ExternalOutput")
        for n in ("im_h", "im_d", "tm_h", "tm_d")
    ]
    # value family: column k*ng + g ; count family likewise
    vacc_d = nc.dram_tensor("vacc", (4, P, _NUM_BINS * ng), f32, kind="ExternalOutput")
    cacc_d = nc.dram_tensor("cacc", (4, P, _NUM_BINS * ng), f32, kind="ExternalOutput")
    bacc_d = nc.dram_tensor("bacc", (4, P, nbands * 4), f32, kind="ExternalOutput")

    cbw = Wimg // 4
    m_h = [np.float32(_RGB_FROM_HED[0, d]) for d in range(3)]
    m_d = [np.float32(_RGB_FROM_HED[2, d]) for d in range(3)]
    lnc = [float(np.float32(math.log(_COEFFS[d]))) for d in range(3)]
    edges = [float(np.float32(k) * _BW32) for k in range(_NUM_BINS)]
    inv_bw = float(1.0 / np.float32(_BW32))

    with tile.TileContext(nc) as tc:
        with (
            tc.tile_pool(name="const", bufs=1) as constp,
            tc.tile_pool(name="chan", bufs=2) as chanp,
            tc.tile_pool(name="epool", bufs=1) as epool,
            tc.tile_pool(name="upool", bufs=1) as upool,
            tc.tile_pool(name="fodw", bufs=2) as fodwp,
            tc.tile_pool(name="codes", bufs=1) as codesp,
            tc.tile_pool(name="maskp", bufs=2) as maskp,
            tc.tile_pool(name="accp", bufs=1) as accp,
            tc.tile_pool(name="scr", bufs=1) as scrp,
            tc.tile_pool(name="psum", bufs=2, space="PSUM") as psump,
        ):
            diags = []
            for i in range(7):
                dt_ = constp.tile([P, P], f32, tag=f"diag{i}", name=f"diag{i}")
                nc.sync.dma_start(dt_[:], diag_d[i])
                diags.append(dt_)
            identr = constp.tile([P, P], f32r, tag="identr", name="identr")
            nc.sync.dma_start(identr[:], identr_d[:])

            vacc_sb = [accp.tile([P, _NUM_BINS * ng], f32, tag=f"vacc{i}",
                                 name=f"vacc{i}") for i in range(4)]
            cacc_sb = [accp.tile([P, _NUM_BINS * ng], f32, tag=f"cacc{i}",
                                 name=f"cacc{i}") for i in range(4)]
            bacc_sb = [accp.tile([P, nbands * 4], f32, tag=f"bacc{i}",
                                 name=f"bacc{i}") for i in range(4)]
            # count column for k=0 is constant N; memset it (skip the pass)
            for si in range(4):
                nc.vector.memset(cacc_sb[si][:, 0:ng], float(wide))

            scr_m = scrp.tile([P, wide], f32, tag="scrm", name="scrm")
            scr_r = scrp.tile([P, wide], f32, tag="scrr", name="scrr")
            scr_c = scrp.tile([P, wide], bf16, tag="scrc", name="scrc")

            bias_lnc = []
            for d in range(3):
                bt = constp.tile([P, 1], f32, tag=f"blnc{d}", name=f"blnc{d}")
                nc.vector.memset(bt[:], lnc[d])
                bias_lnc.append(bt)
            bias_cal = constp.tile([P, 1], f32)
            nc.vector.memset(bias_cal[:], float(_CAL))
            bias_rk = {}
            for k in ACT_BINS:
                bt = constp.tile([P, 1], f32, tag=f"brk{k}", name=f"brk{k}")
                nc.vector.memset(bt[:], -edges[k])
                bias_rk[k] = bt

            for img_i, img_d in ((0, x_d), (1, y_d)):
                for g in range(ng):
                    fodw = [fodwp.tile([P, wide], f32, tag=f"fodw{st}",
                                       name=f"fodw{st}") for st in range(2)]
                    codesw = [codesp.tile([P, wide], bf16, tag=f"codes{st}",
                                          name=f"codes{st}") for st in range(2)]
                    for bi in range(wg):
                        b = g * wg + bi
                        rows = slice(b * P, (b + 1) * P)
                        wcols = slice(bi * Wimg, (bi + 1) * Wimg)
                        L = []
                        for c in range(3):
                            t = chanp.tile([P, Wimg], f32, tag=f"ch{c}",
                                           name=f"ch{c}")
                            nc.sync.dma_start(t[:], img_d[c, rows, :])
                            nc.vector.tensor_scalar_max(t[:], t[:], 1e-6)
                            nc.scalar.activation(
                                t[:], t[:], mybir.ActivationFunctionType.Ln
                            )
                            L.append(t)

                        for st in range(2):
                            si = img_i * 2 + st
                            mvals = m_h if st == 0 else m_d
                            sp = psump.tile([P, Wimg], f32, tag="sp")
                            for c2 in range(Wimg // mmchunk):
                                cols = slice(c2 * mmchunk, (c2 + 1) * mmchunk)
                                for c in range(3):
                                    nc.tensor.matmul(
                                        sp[:, cols],
                                        diags[3 * st + c][:],
                                        L[c][:, cols],
                                        start=(c == 0),
                                        stop=(c == 2),
                                    )
                            E = []
                            for d in range(3):
                                e = epool.tile([P, Wimg],
                                               f32r if USE_F32R else f32,
                                               tag=f"e{d}", name=f"e{d}")
                                nc.scalar.activation(
                                    e[:],
                                    sp[:],
                                    mybir.ActivationFunctionType.Exp,
                                    bias=bias_lnc[d][:],
                                    scale=float(mvals[d]),
                                )
                                E.append(e)
                            G = psump.tile([P, Wimg], f32, tag="G")
                            for c2 in range(Wimg // mmchunk):
                                cols = slice(c2 * mmchunk, (c2 + 1) * mmchunk)
                                for d in range(3):
                                    nc.tensor.matmul(
                                        G[:, cols],
                                        identr[:] if USE_F32R else diags[6][:],
                                        E[d][:, cols],
                                        start=(d == 0),
                                        stop=(d == 2),
                                    )
                            u = upool.tile([P, Wimg], f32)
                            nc.scalar.activation(
                                u[:], G[:], mybir.ActivationFunctionType.Ln,
                                bias=bias_cal[:],
                            )
                            fod = fodw[st][:, wcols]
                            nc.vector.tensor_scalar(
                                fod, u[:], -float(_INV_LN10), 0.0,
                                mybir.AluOpType.mult, mybir.AluOpType.max,
                            )
                            mk = maskp.tile([P, Wimg], f32)
                            nc.vector.tensor_scalar(
                                mk[:], fod, float(np.float32(_T_MASK)), None,
                                mybir.AluOpType.is_ge,
                            )
                            nc.sync.dma_start(masks_d[si][rows, :], mk[:])
                            for cb in range(4):
                                ccols = slice(bi * Wimg + cb * cbw,
                                              bi * Wimg + (cb + 1) * cbw)
                                nc.vector.scalar_tensor_tensor(
                                    scr_m[:, 0:cbw],
                                    fodw[st][:, ccols],
                                    float(np.float32(_T_FOD)),
                                    fodw[st][:, ccols],
                                    mybir.AluOpType.is_ge,
                                    mybir.AluOpType.mult,
                                    accum_out=bacc_sb[si][
                                        :, b * 4 + cb : b * 4 + cb + 1],
                                )
                            # codes: RTNE(fod/bw - 0.5) == floor(fod/bw) on HW
                            ci = codesp.tile([P, Wimg], i32, tag="ci", name="ci")
                            nc.vector.tensor_scalar(
                                ci[:], fod, inv_bw, 0.5,
                                mybir.AluOpType.mult, mybir.AluOpType.subtract,
                            )
                            nc.vector.tensor_copy(codesw[st][:, wcols], ci[:])

                    # wide-group histogram passes
                    for st in range(2):
                        si = img_i * 2 + st
                        for k in range(_NUM_BINS):
                            col = k * ng + g
                            if k > 0:
                                nc.vector.tensor_scalar(
                                    scr_c[:], codesw[st][:],
                                    float(k) - 0.5, None,
                                    mybir.AluOpType.is_ge,
                                    op1=mybir.AluOpType.add,
                                    accum_out=cacc_sb[si][:, col:col + 1],
                                )
                            if k in ACT_BINS:
                                nc.scalar.activation(
                                    scr_r[:], fodw[st][:],
                                    mybir.ActivationFunctionType.Relu,
                                    bias=bias_rk[k][:],
                                    accum_out=vacc_sb[si][:, col:col + 1],
                                )
                            else:
                                nc.vector.tensor_scalar(
                                    scr_m[:], fodw[st][:],
                                    edges[k], None,
                                    mybir.AluOpType.max,
                                    op1=mybir.AluOpType.add,
                                    accum_out=vacc_sb[si][:, col:col + 1],
                                )

            for si in range(4):
                nc.sync.dma_start(vacc_d[si], vacc_sb[si][:])
                nc.sync.dma_start(cacc_d[si], cacc_sb[si][:])
                nc.sync.dma_start(bacc_d[si], bacc_sb[si][:])

    return dict(
        inputs=("x", "y", "diags", "identr"),
        outputs=("im_h", "im_d", "tm_h", "tm_d", "vacc", "cacc", "bacc"),
        nbands=nbands,
        ng=ng,
        wide=wide,
    )


# --------------------------------------------------------------------------
# host-side finishing
# --------------------------------------------------------------------------


def _finish_stats(vacc, cacc, bacc, nbands: int, ng: int, Wimg: int = W):
    """-> per stain-image: hist [20], blocks [4,4], avg (f64)."""
    Himg = nbands * P
    rows_per_block = Himg // 4
    grow = np.arange(nbands * P) // rows_per_block
    edges64 = np.float64([np.float32(k) * _BW32 for k in range(_NUM_BINS)])
    hists, blocks, avgs = [], [], []
    for si in range(4):
        V = vacc[si].reshape(P, _NUM_BINS, ng).sum(axis=(0, 2), dtype=np.float64)
        Cnt = cacc[si].reshape(P, _NUM_BINS, ng).sum(axis=(0, 2), dtype=np.float64)
        Ntot = float(Himg) * float(Wimg)
        M = np.zeros(_NUM_BINS)
        for k in range(_NUM_BINS):
            r_k = V[k] - (0.0 if k in ACT_BINS else edges64[k] * Ntot)
            M[k] = r_k + edges64[k] * Cnt[k]
        bins = M.copy()
        bins[:-1] -= M[1:]
        hists.append(bins)
        per_row = bacc[si].reshape(P, nbands, 4).transpose(1, 0, 2).reshape(-1, 4)
        blk = np.zeros((4, 4))
        for r in range(4):
            blk[r] = per_row[grow == r].sum(axis=0, dtype=np.float64)
        blocks.append(blk)
        avgs.append(blk.sum())
    return hists, blocks, avgs


def _channel_loss(i_avg, i_blk, i_his, t_avg, t_blk, t_his, Bsz, HWsz):
    avg_t = (i_avg - t_avg) ** 2 / float(HWsz) ** 2
    his_t = np.sum((i_his / HWsz - t_his / HWsz) ** 2, axis=1) / Bsz
    blk_t = np.mean((i_blk / (HWsz / 16.0) - t_blk / (HWsz / 16.0)) ** 2)
    diff = i_avg - t_avg
    cond = (diff >= t_avg * -0.4) & (diff <= t_avg * 0.4)
    return np.sum(np.where(cond, his_t, avg_t + his_t)) + blk_t


_BUILT = {}
LAST_RESULTS = None


def _get_compiled():
    key = (H, W)
    if key not in _BUILT:
        nc = bacc.Bacc("TRN2", target_bir_lowering=False, debug=False)
        info = build_program(nc, H, W)
        nc.compile()
        _BUILT[key] = (nc, info)
    return _BUILT[key]


def kernel(inputs: np.ndarray, targets: np.ndarray):
    inputs = np.ascontiguousarray(np.asarray(inputs, dtype=np.float32))
    targets = np.ascontiguousarray(np.asarray(targets, dtype=np.float32))
    assert inputs.shape == (B, C, H, W)

    nc, info = _get_compiled()
    diags = _diag_consts()
    in_maps = [
        {"x": inputs[b], "y": targets[b], "diags": diags[:7],
         "identr": diags[7]} for b in range(B)
    ]
    trace = bool(int(os.environ.get("TRN_KERNEL_TRACE", "0")))
    res = bass_utils.run_bass_kernel_spmd(
        nc, in_maps, core_ids=list(range(B)), trace=trace
    )
    global LAST_RESULTS
    LAST_RESULTS = res
    results = res.results

    nbands, ng = H // P, info["ng"]
    im_h = np.stack([results[b]["im_h"] for b in range(B)])
    im_d = np.stack([results[b]["im_d"] for b in range(B)])
    tm_h = np.stack([results[b]["tm_h"] for b in range(B)])
    tm_d = np.stack([results[b]["tm_d"] for b in range(B)])

    ia = np.zeros((4, B)); ih = np.zeros((4, B, _NUM_BINS)); ib = np.zeros((4, B, 4, 4))
    for b in range(B):
        hists, blocks, avgs = _finish_stats(
            results[b]["vacc"], results[b]["cacc"], results[b]["bacc"],
            nbands, ng,
        )
        for si in range(4):
            ia[si, b] = avgs[si]
            ih[si, b] = hists[si]
            ib[si, b] = blocks[si]

    HWsz = H * W
    loss = _channel_loss(ia[0], ib[0], ih[0], ia[2], ib[2], ih[2], B, HWsz) + \
        _channel_loss(ia[1], ib[1], ih[1], ia[3], ib[3], ih[3], B, HWsz)

    return (np.float32(loss), im_h, tm_h, im_d, tm_d)


# revision 16
# speedup vs baseline: 1.7699x; 1.7699x over previous
"""Trainium2 Bass kernel for nn_DCP_LOSS (stain-deconvolution loss).

Data-parallel over batch: B=8 -> 8 NeuronCores, one batch item per core.
Per core, for its (input, target) pair and both stains (h, d):
  ln(clamped rgb) -> channel mix via PE diag-matmuls -> 3x exp (coeffs folded
  into exp bias) -> PE identity-accumulate (f32r) -> ln(G+calib) -> fod.
  Outputs: masks (fod >= 0.3), block sums of fod_relu, and a 20-bin weighted
  histogram recovered from M_k = sum fod*(fod >= t_k) via
    value family  V_k = sum max(fod, t_k)        (DVE, f32 2x)  == r_k + t_k*N
              or  r_k = sum relu(fod - t_k)      (ACT, accum)
    count family  C_k = sum (code >= k)          (DVE, bf16 4x on int codes)
    M_k = r_k + t_k*C_k ;  bin_k = M_k - M_{k+1}
Host combines the tiny per-core stats into the final scalar loss.

The math restructuring (verified exact): all reference clips only matter
where fod == 0, which contributes 0 to every reduction, so
fod = max(-log10(sum_d exp(M[row,d]*s' + ln c_d) + calib), 0) with
s' = sum_c HED[c,col]*ln(max(rgb_c, 1e-6)).
"""

import math
import os
import sys

sys.path.insert(0, "/opt/trn_rl_repo")

import numpy as np


def _setup_act_tables():
    """Point walrus at an act_info.json whose first set holds BOTH ln and
    exp, so the per-band ln/exp interleave does not thrash ACT_TABLE_LOAD."""
    if not os.environ.get("DCP_ACT_TABLE_FIX"):  # crashes remote NRT; keep off
        return
    if os.environ.get("BASS_ACT_ROOT_JSON_PATH"):
        return
    import glob
    import json

    cands = glob.glob(
        "/nix/store/*/lib/python3.13/site-packages/neuronxcc/pwp/"
        "pwp_bin_trainium/act_info.json"
    )
    if not cands:
        return
    src = cands[0]
    srcdir = os.path.dirname(src)
    dst = "/tmp/dcp_pwp"
    os.makedirs(dst, exist_ok=True)
    for f in os.listdir(srcdir):
        t = os.path.join(dst, f)
        if not os.path.exists(t):
            try:
                os.symlink(os.path.join(srcdir, f), t)
            except OSError:
                pass
    d = json.load(open(src))
    sets = d["act_func_sets"]
    first = [s for s in sets if s["name"] == "natural_log_exp_and_others"]
    rest = [s for s in sets if s["name"] != "natural_log_exp_and_others"]
    d["act_func_sets"] = first + rest
    out = os.path.join(dst, "act_info.json")
    os.unlink(out) if os.path.islink(out) else None
    with open(out, "w") as f:
        json.dump(d, f)
    os.environ["BASS_ACT_ROOT_JSON_PATH"] = out


_setup_act_tables()

import concourse.bacc as bacc
import concourse.bass as bass
import concourse.mybir as mybir
import concourse.tile as tile
from concourse import bass_utils

f32 = mybir.dt.float32
f32r = mybir.dt.float32r
bf16 = mybir.dt.bfloat16
i32 = mybir.dt.int32

# ---- constants (from the reference) ----
_RGB_FROM_HED = np.array(
    [[0.65, 0.7, 0.29], [0.07, 0.99, 0.11], [0.27, 0.57, 0.78]], dtype=np.float64
)
_HED_FROM_RGB = np.linalg.inv(_RGB_FROM_HED)
_COEFFS = np.array([0.2125, 0.7154, 0.0721], dtype=np.float64)
_CAL = 10.0 ** (-math.e)
_T_FOD = 0.15
_T_MASK = 0.3
_NUM_BINS = 20
_BW32 = np.float32(math.e) / np.float32(20.0)
_INV_LN10 = 1.0 / math.log(10.0)

B, C, H, W = 8, 3, 1024, 1024
P = 128

# bins whose value-measurement runs on ACT (r_k = sum relu(fod-t_k));
# the rest run on DVE (V_k = sum max(fod, t_k)).
ACT_BINS = (() if os.environ.get("DCP_NO_ACT_ACCUM")
            else tuple(int(s) for s in os.environ.get(
                "DCP_ACT_BINS", "14,15,16,17,18,19").split(",") if s))
USE_F32R = not os.environ.get("DCP_NO_F32R")
WG = 4  # bands per wide histogram group


def _diag_consts() -> np.ndarray:
    out = np.zeros((8, P, P), dtype=np.float32)
    eye = np.eye(P, dtype=np.float32)
    for c in range(3):
        out[c] = np.float32(_HED_FROM_RGB[c, 0]) * eye
        out[3 + c] = np.float32(_HED_FROM_RGB[c, 2]) * eye
    out[6] = eye
    out[7] = eye  # f32r identity (same bits)
    return out


def build_program(nc, Himg: int, Wimg: int):
    nbands = Himg // P
    mmchunk = min(512, Wimg)
    wg = min(WG, nbands)
    ng = nbands // wg
    assert Himg % P == 0 and Wimg % mmchunk == 0 and nbands % wg == 0
    wide = wg * Wimg

    x_d = nc.dram_tensor("x", (3, Himg, Wimg), f32, kind="ExternalInput")
    y_d = nc.dram_tensor("y", (3, Himg, Wimg), f32, kind="ExternalInput")
    diag_d = nc.dram_tensor("diags", (7, P, P), f32, kind="ExternalInput")
    identr_d = nc.dram_tensor("identr", (P, P), f32r, kind="ExternalInput")

    masks_d = [
        nc.dram_tensor(n, (Himg, Wimg), f32, kind="ExternalOutput")
        for n in ("im_h", "im_d", "tm_h", "tm_d")
    ]
    # value family: column k*ng + g ; count family likewise
    vacc_d = nc.dram_tensor("vacc", (4, P, _NUM_BINS * ng), f32, kind="ExternalOutput")
    cacc_d = nc.dram_tensor("cacc", (4, P, _NUM_BINS * ng), f32, kind="ExternalOutput")
    bacc_d = nc.dram_tensor("bacc", (4, P, nbands * 4), f32, kind="ExternalOutput")

    cbw = Wimg // 4
    m_h = [np.float32(_RGB_FROM_HED[0, d]) for d in range(3)]
    m_d = [np.float32(_RGB_FROM_HED[2, d]) for d in range(3)]
    lnc = [float(np.float32(math.log(_COEFFS[d]))) for d in range(3)]
    edges = [float(np.float32(k) * _BW32) for k in range(_NUM_BINS)]
    inv_bw = float(1.0 / np.float32(_BW32))

    with tile.TileContext(nc) as tc:
        with (
            tc.tile_pool(name="const", bufs=1) as constp,
            tc.tile_pool(name="chan", bufs=2) as chanp,
            tc.tile_pool(name="epool", bufs=1) as epool,
            tc.tile_pool(name="upool", bufs=1) as upool,
            tc.tile_pool(name="fodw", bufs=2) as fodwp,
            tc.tile_pool(name="codes", bufs=1) as codesp,
            tc.tile_pool(name="maskp", bufs=2) as maskp,
            tc.tile_pool(name="accp", bufs=1) as accp,
            tc.tile_pool(name="scr", bufs=1) as scrp,
            tc.tile_pool(name="psum", bufs=2, space="PSUM") as psump,
        ):
            diags = []
            for i in range(7):
                dt_ = constp.tile([P, P], f32, tag=f"diag{i}", name=f"diag{i}")
                nc.sync.dma_start(dt_[:], diag_d[i])
                diags.append(dt_)
            identr = constp.tile([P, P], f32r, tag="identr", name="identr")
            nc.sync.dma_start(identr[:], identr_d[:])

            vacc_sb = [accp.tile([P, _NUM_BINS * ng], f32, tag=f"vacc{i}",
                                 name=f"vacc{i}") for i in range(4)]
            cacc_sb = [accp.tile([P, _NUM_BINS * ng], f32, tag=f"cacc{i}",
                                 name=f"cacc{i}") for i in range(4)]
            bacc_sb = [accp.tile([P, nbands * 4], f32, tag=f"bacc{i}",
                                 name=f"bacc{i}") for i in range(4)]
            for si in range(4):
                nc.vector.memset(cacc_sb[si][:], 0.0)

            scr_m = scrp.tile([P, wide], f32, tag="scrm", name="scrm")
            scr_r = scrp.tile([P, wide], f32, tag="scrr", name="scrr")

            bias_lnc = []
            for d in range(3):
                bt = constp.tile([P, 1], f32, tag=f"blnc{d}", name=f"blnc{d}")
                nc.vector.memset(bt[:], lnc[d])
                bias_lnc.append(bt)
            bias_cal = constp.tile([P, 1], f32)
            nc.vector.memset(bias_cal[:], float(_CAL))
            bias_rk = {}
            bias_sk = {}
            for k in ACT_BINS:
                bt = constp.tile([P, 1], f32, tag=f"brk{k}", name=f"brk{k}")
                nc.vector.memset(bt[:], -edges[k])
                bias_rk[k] = bt
                bs = constp.tile([P, 1], f32, tag=f"bsk{k}", name=f"bsk{k}")
                bk = float(np.nextafter(np.float32(edges[k]), np.float32(-1.0)))
                nc.vector.memset(bs[:], -bk)
                bias_sk[k] = bs

            for img_i, img_d in ((0, x_d), (1, y_d)):
                for g in range(ng):
                    fodw = [fodwp.tile([P, wide], f32, tag=f"fodw{st}",
                                       name=f"fodw{st}") for st in range(2)]
                    for bi in range(wg):
                        b = g * wg + bi
                        rows = slice(b * P, (b + 1) * P)
                        wcols = slice(bi * Wimg, (bi + 1) * Wimg)
                        L = []
                        for c in range(3):
                            t = chanp.tile([P, Wimg], f32, tag=f"ch{c}",
                                           name=f"ch{c}")
                            nc.sync.dma_start(t[:], img_d[c, rows, :])
                            nc.vector.tensor_scalar_max(t[:], t[:], 1e-6)
                            nc.scalar.activation(
                                t[:], t[:], mybir.ActivationFunctionType.Ln
                            )
                            L.append(t)

                        for st in range(2):
                            si = img_i * 2 + st
                            mvals = m_h if st == 0 else m_d
                            sp = psump.tile([P, Wimg], f32, tag="sp")
                            for c2 in range(Wimg // mmchunk):
                                cols = slice(c2 * mmchunk, (c2 + 1) * mmchunk)
                                for c in range(3):
                                    nc.tensor.matmul(
                                        sp[:, cols],
                                        diags[3 * st + c][:],
                                        L[c][:, cols],
                                        start=(c == 0),
                                        stop=(c == 2),
                                    )
                            E = []
                            for d in range(3):
                                e = epool.tile([P, Wimg],
                                               f32r if USE_F32R else f32,
                                               tag=f"e{d}", name=f"e{d}")
                                nc.scalar.activation(
                                    e[:],
                                    sp[:],
                                    mybir.ActivationFunctionType.Exp,
                                    bias=bias_lnc[d][:],
                                    scale=float(mvals[d]),
                                )
                                E.append(e)
                            G = psump.tile([P, Wimg], f32, tag="G")
                            for c2 in range(Wimg // mmchunk):
                                cols = slice(c2 * mmchunk, (c2 + 1) * mmchunk)
                                for d in range(3):
                                    nc.tensor.matmul(
                                        G[:, cols],
                                        identr[:] if USE_F32R else diags[6][:],
                                        E[d][:, cols],
                                        start=(d == 0),
                                        stop=(d == 2),
                                    )
                            u = upool.tile([P, Wimg], f32)
                            nc.scalar.activation(
                                u[:], G[:], mybir.ActivationFunctionType.Ln,
                                bias=bias_cal[:],
                            )
                            fod = fodw[st][:, wcols]
                            nc.vector.tensor_scalar(
                                fod, u[:], -float(_INV_LN10), 0.0,
                                mybir.AluOpType.mult, mybir.AluOpType.max,
                            )
                            mk = maskp.tile([P, Wimg], f32)
                            nc.vector.tensor_scalar(
                                mk[:], fod, float(np.float32(_T_MASK)), None,
                                mybir.AluOpType.is_ge,
                            )
                            nc.sync.dma_start(masks_d[si][rows, :], mk[:])
                            for cb in range(4):
                                ccols = slice(bi * Wimg + cb * cbw,
                                              bi * Wimg + (cb + 1) * cbw)
                                nc.vector.scalar_tensor_tensor(
                                    scr_m[:, 0:cbw],
                                    fodw[st][:, ccols],
                                    float(np.float32(_T_FOD)),
                                    fodw[st][:, ccols],
                                    mybir.AluOpType.is_ge,
                                    mybir.AluOpType.mult,
                                    accum_out=bacc_sb[si][
                                        :, b * 4 + cb : b * 4 + cb + 1],
                                )

                    # wide-group histogram passes
                    for st in range(2):
                        si = img_i * 2 + st
                        for k in range(_NUM_BINS):
                            col = k * ng + g
                            if k in ACT_BINS:
                                nc.scalar.activation(
                                    scr_r[:], fodw[st][:],
                                    mybir.ActivationFunctionType.Relu,
                                    bias=bias_rk[k][:],
                                    accum_out=vacc_sb[si][:, col:col + 1],
                                )
                                nc.scalar.activation(
                                    scr_r[:], fodw[st][:],
                                    mybir.ActivationFunctionType.Sign,
                                    bias=bias_sk[k][:],
                                    accum_out=cacc_sb[si][:, col:col + 1],
                                )
                            else:
                                nc.vector.scalar_tensor_tensor(
                                    scr_m[:], fodw[st][:],
                                    edges[k],
                                    fodw[st][:],
                                    mybir.AluOpType.is_ge,
                                    mybir.AluOpType.mult,
                                    accum_out=vacc_sb[si][:, col:col + 1],
                                )

            for si in range(4):
                nc.sync.dma_start(vacc_d[si], vacc_sb[si][:])
                nc.sync.dma_start(cacc_d[si], cacc_sb[si][:])
                nc.sync.dma_start(bacc_d[si], bacc_sb[si][:])

    return dict(
        inputs=("x", "y", "diags", "identr"),
        outputs=("im_h", "im_d", "tm_h", "tm_d", "vacc", "cacc", "bacc"),
        nbands=nbands,
        ng=ng,
        wide=wide,
    )


# --------------------------------------------------------------------------
# host-side finishing
# --------------------------------------------------------------------------


def _finish_stats(vacc, cacc, bacc, nbands: int, ng: int, Wimg: int = W):
    """-> per stain-image: hist [20], blocks [4,4], avg (f64)."""
    Himg = nbands * P
    rows_per_block = Himg // 4
    grow = np.arange(nbands * P) // rows_per_block
    edges64 = np.float64([np.float32(k) * _BW32 for k in range(_NUM_BINS)])
    hists, blocks, avgs = [], [], []
    for si in range(4):
        V = vacc[si].reshape(P, _NUM_BINS, ng).sum(axis=(0, 2), dtype=np.float64)
        Sg = cacc[si].reshape(P, _NUM_BINS, ng).sum(axis=(0, 2), dtype=np.float64)
        Ntot = float(Himg) * float(Wimg)
        M = np.zeros(_NUM_BINS)
        for k in range(_NUM_BINS):
            if k in ACT_BINS:
                C_k = (Sg[k] + Ntot) / 2.0
                M[k] = V[k] + edges64[k] * C_k
            else:
                M[k] = V[k]
        bins = M.copy()
        bins[:-1] -= M[1:]
        hists.append(bins)
        per_row = bacc[si].reshape(P, nbands, 4).transpose(1, 0, 2).reshape(-1, 4)
        blk = np.zeros((4, 4))
        for r in range(4):
            blk[r] = per_row[grow == r].sum(axis=0, dtype=np.float64)
        blocks.append(blk)
        avgs.append(blk.sum())
    return hists, blocks, avgs


def _channel_loss(i_avg, i_blk, i_his, t_avg, t_blk, t_his, Bsz, HWsz):
    avg_t = (i_avg - t_avg) ** 2 / float(HWsz) ** 2
    his_t = np.sum((i_his / HWsz - t_his / HWsz) ** 2, axis=1) / Bsz
    blk_t = np.mean((i_blk / (HWsz / 16.0) - t_blk / (HWsz / 16.0)) ** 2)
    diff = i_avg - t_avg
    cond = (diff >= t_avg * -0.4) & (diff <= t_avg * 0.4)
    return np.sum(np.where(cond, his_t, avg_t + his_t)) + blk_t


_BUILT = {}
LAST_RESULTS = None


def _get_compiled():
    key = (H, W)
    if key not in _BUILT:
        nc = bacc.Bacc("TRN2", target_bir_lowering=False, debug=False)
        info = build_program(nc, H, W)
        nc.compile()
        _BUILT[key] = (nc, info)
    return _BUILT[key]


def kernel(inputs: np.ndarray, targets: np.ndarray):
    inputs = np.ascontiguousarray(np.asarray(inputs, dtype=np.float32))
    targets = np.ascontiguousarray(np.asarray(targets, dtype=np.float32))
    assert inputs.shape == (B, C, H, W)

    nc, info = _get_compiled()
    diags = _diag_consts()
    in_maps = [
        {"x": inputs[b], "y": targets[b], "diags": diags[:7],
         "identr": diags[7]} for b in range(B)
    ]
    trace = bool(int(os.environ.get("TRN_KERNEL_TRACE", "0")))
    res = bass_utils.run_bass_kernel_spmd(
        nc, in_maps, core_ids=list(range(B)), trace=trace
    )
    global LAST_RESULTS
    LAST_RESULTS = res
    results = res.results

    nbands, ng = H // P, info["ng"]
    im_h = np.stack([results[b]["im_h"] for b in range(B)])
    im_d = np.stack([results[b]["im_d"] for b in range(B)])
    tm_h = np.stack([results[b]["tm_h"] for b in range(B)])
    tm_d = np.stack([results[b]["tm_d"] for b in range(B)])

    ia = np.zeros((4, B)); ih = np.zeros((4, B, _NUM_BINS)); ib = np.zeros((4, B, 4, 4))
    for b in range(B):
        hists, blocks, avgs = _finish_stats(
            results[b]["vacc"], results[b]["cacc"], results[b]["bacc"],
            nbands, ng,
        )
        for si in range(4):
            ia[si, b] = avgs[si]
            ih[si, b] = hists[si]
            ib[si, b] = blocks[si]

    HWsz = H * W
    loss = _channel_loss(ia[0], ib[0], ih[0], ia[2], ib[2], ih[2], B, HWsz) + \
        _channel_loss(ia[1], ib[1], ih[1], ia[3], ib[3], ih[3], B, HWsz)

    return (np.float32(loss), im_h, tm_h, im_d, tm_d)


# revision 17
# speedup vs baseline: 1.8467x; 1.0434x over previous
"""Trainium2 Bass kernel for nn_DCP_LOSS (stain-deconvolution loss).

Data-parallel over batch: B=8 -> 8 NeuronCores, one batch item per core.
Per core, for its (input, target) pair and both stains (h, d):
  ln(clamped rgb) -> channel mix via PE diag-matmuls -> 3x exp (coeffs folded
  into exp bias) -> PE identity-accumulate (f32r) -> ln(G+calib) -> fod.
  Outputs: masks (fod >= 0.3), block sums of fod_relu, and a 20-bin weighted
  histogram recovered from M_k = sum fod*(fod >= t_k) via
    value family  V_k = sum max(fod, t_k)        (DVE, f32 2x)  == r_k + t_k*N
              or  r_k = sum relu(fod - t_k)      (ACT, accum)
    count family  C_k = sum (code >= k)          (DVE, bf16 4x on int codes)
    M_k = r_k + t_k*C_k ;  bin_k = M_k - M_{k+1}
Host combines the tiny per-core stats into the final scalar loss.

The math restructuring (verified exact): all reference clips only matter
where fod == 0, which contributes 0 to every reduction, so
fod = max(-log10(sum_d exp(M[row,d]*s' + ln c_d) + calib), 0) with
s' = sum_c HED[c,col]*ln(max(rgb_c, 1e-6)).
"""

import math
import os
import sys

sys.path.insert(0, "/opt/trn_rl_repo")

import numpy as np


def _setup_act_tables():
    """Point walrus at an act_info.json whose first set holds BOTH ln and
    exp, so the per-band ln/exp interleave does not thrash ACT_TABLE_LOAD."""
    if not os.environ.get("DCP_ACT_TABLE_FIX"):  # crashes remote NRT; keep off
        return
    if os.environ.get("BASS_ACT_ROOT_JSON_PATH"):
        return
    import glob
    import json

    cands = glob.glob(
        "/nix/store/*/lib/python3.13/site-packages/neuronxcc/pwp/"
        "pwp_bin_trainium/act_info.json"
    )
    if not cands:
        return
    src = cands[0]
    srcdir = os.path.dirname(src)
    dst = "/tmp/dcp_pwp"
    os.makedirs(dst, exist_ok=True)
    for f in os.listdir(srcdir):
        t = os.path.join(dst, f)
        if not os.path.exists(t):
            try:
                os.symlink(os.path.join(srcdir, f), t)
            except OSError:
                pass
    d = json.load(open(src))
    sets = d["act_func_sets"]
    first = [s for s in sets if s["name"] == "natural_log_exp_and_others"]
    rest = [s for s in sets if s["name"] != "natural_log_exp_and_others"]
    d["act_func_sets"] = first + rest
    out = os.path.join(dst, "act_info.json")
    os.unlink(out) if os.path.islink(out) else None
    with open(out, "w") as f:
        json.dump(d, f)
    os.environ["BASS_ACT_ROOT_JSON_PATH"] = out


_setup_act_tables()

import concourse.bacc as bacc
import concourse.bass as bass
import concourse.mybir as mybir
import concourse.tile as tile
from concourse import bass_utils

f32 = mybir.dt.float32
f32r = mybir.dt.float32r
bf16 = mybir.dt.bfloat16
i32 = mybir.dt.int32

# ---- constants (from the reference) ----
_RGB_FROM_HED = np.array(
    [[0.65, 0.7, 0.29], [0.07, 0.99, 0.11], [0.27, 0.57, 0.78]], dtype=np.float64
)
_HED_FROM_RGB = np.linalg.inv(_RGB_FROM_HED)
_COEFFS = np.array([0.2125, 0.7154, 0.0721], dtype=np.float64)
_CAL = 10.0 ** (-math.e)
_T_FOD = 0.15
_T_MASK = 0.3
_NUM_BINS = 20
_BW32 = np.float32(math.e) / np.float32(20.0)
_INV_LN10 = 1.0 / math.log(10.0)

B, C, H, W = 8, 3, 1024, 1024
P = 128

# bins whose value-measurement runs on ACT (r_k = sum relu(fod-t_k));
# the rest run on DVE (V_k = sum max(fod, t_k)).
ACT_BINS = (() if os.environ.get("DCP_NO_ACT_ACCUM")
            else tuple(int(s) for s in os.environ.get(
                "DCP_ACT_BINS", "14,15,16,17,18,19").split(",") if s))
USE_F32R = bool(os.environ.get("DCP_F32R"))
WG = 4  # bands per wide histogram group


def _diag_consts() -> np.ndarray:
    out = np.zeros((8, P, P), dtype=np.float32)
    eye = np.eye(P, dtype=np.float32)
    for c in range(3):
        out[c] = np.float32(_HED_FROM_RGB[c, 0]) * eye
        out[3 + c] = np.float32(_HED_FROM_RGB[c, 2]) * eye
    out[6] = eye
    out[7] = eye  # f32r identity (same bits)
    return out


def build_program(nc, Himg: int, Wimg: int):
    nbands = Himg // P
    mmchunk = min(512, Wimg)
    wg = min(WG, nbands)
    ng = nbands // wg
    assert Himg % P == 0 and Wimg % mmchunk == 0 and nbands % wg == 0
    wide = wg * Wimg

    x_d = nc.dram_tensor("x", (3, Himg, Wimg), f32, kind="ExternalInput")
    y_d = nc.dram_tensor("y", (3, Himg, Wimg), f32, kind="ExternalInput")
    diag_d = nc.dram_tensor("diags", (7, P, P), f32, kind="ExternalInput")
    identr_d = nc.dram_tensor("identr", (P, P), f32r, kind="ExternalInput")

    masks_d = [
        nc.dram_tensor(n, (Himg, Wimg), f32, kind="ExternalOutput")
        for n in ("im_h", "im_d", "tm_h", "tm_d")
    ]
    # value family: column k*ng + g ; count family likewise
    vacc_d = nc.dram_tensor("vacc", (4, P, _NUM_BINS * ng), f32, kind="ExternalOutput")
    cacc_d = nc.dram_tensor("cacc", (4, P, _NUM_BINS * ng), f32, kind="ExternalOutput")
    bacc_d = nc.dram_tensor("bacc", (4, P, nbands * 4), f32, kind="ExternalOutput")

    cbw = Wimg // 4
    m_h = [np.float32(_RGB_FROM_HED[0, d]) for d in range(3)]
    m_d = [np.float32(_RGB_FROM_HED[2, d]) for d in range(3)]
    lnc = [float(np.float32(math.log(_COEFFS[d]))) for d in range(3)]
    edges = [float(np.float32(k) * _BW32) for k in range(_NUM_BINS)]
    inv_bw = float(1.0 / np.float32(_BW32))

    with tile.TileContext(nc) as tc:
        with (
            tc.tile_pool(name="const", bufs=1) as constp,
            tc.tile_pool(name="chan", bufs=2) as chanp,
            tc.tile_pool(name="epool", bufs=1) as epool,
            tc.tile_pool(name="upool", bufs=1) as upool,
            tc.tile_pool(name="fodw", bufs=2) as fodwp,
            tc.tile_pool(name="codes", bufs=1) as codesp,
            tc.tile_pool(name="maskp", bufs=2) as maskp,
            tc.tile_pool(name="accp", bufs=1) as accp,
            tc.tile_pool(name="scr", bufs=1) as scrp,
            tc.tile_pool(name="psum", bufs=1, space="PSUM") as psump,
        ):
            diags = []
            for i in range(7):
                dt_ = constp.tile([P, P], f32, tag=f"diag{i}", name=f"diag{i}")
                nc.sync.dma_start(dt_[:], diag_d[i])
                diags.append(dt_)
            identr = constp.tile([P, P], f32r, tag="identr", name="identr")
            nc.sync.dma_start(identr[:], identr_d[:])

            vacc_sb = [accp.tile([P, _NUM_BINS * ng], f32, tag=f"vacc{i}",
                                 name=f"vacc{i}") for i in range(4)]
            cacc_sb = [accp.tile([P, _NUM_BINS * ng], f32, tag=f"cacc{i}",
                                 name=f"cacc{i}") for i in range(4)]
            bacc_sb = [accp.tile([P, nbands * 4], f32, tag=f"bacc{i}",
                                 name=f"bacc{i}") for i in range(4)]
            for si in range(4):
                nc.vector.memset(cacc_sb[si][:], 0.0)

            scr_m = scrp.tile([P, wide], f32, tag="scrm", name="scrm")
            scr_r = scrp.tile([P, wide], f32, tag="scrr", name="scrr")

            bias_lnc = []
            for d in range(3):
                bt = constp.tile([P, 1], f32, tag=f"blnc{d}", name=f"blnc{d}")
                nc.vector.memset(bt[:], lnc[d])
                bias_lnc.append(bt)
            bias_cal = constp.tile([P, 1], f32)
            nc.vector.memset(bias_cal[:], float(_CAL))
            bias_rk = {}
            bias_sk = {}
            for k in ACT_BINS:
                bt = constp.tile([P, 1], f32, tag=f"brk{k}", name=f"brk{k}")
                nc.vector.memset(bt[:], -edges[k])
                bias_rk[k] = bt
                bs = constp.tile([P, 1], f32, tag=f"bsk{k}", name=f"bsk{k}")
                bk = float(np.nextafter(np.float32(edges[k]), np.float32(-1.0)))
                nc.vector.memset(bs[:], -bk)
                bias_sk[k] = bs

            for img_i, img_d in ((0, x_d), (1, y_d)):
                for g in range(ng):
                    fodw = [fodwp.tile([P, wide], f32, tag=f"fodw{st}",
                                       name=f"fodw{st}") for st in range(2)]
                    for bi in range(wg):
                        b = g * wg + bi
                        rows = slice(b * P, (b + 1) * P)
                        wcols = slice(bi * Wimg, (bi + 1) * Wimg)
                        L = []
                        for c in range(3):
                            t = chanp.tile([P, Wimg], f32, tag=f"ch{c}",
                                           name=f"ch{c}")
                            nc.sync.dma_start(t[:], img_d[c, rows, :])
                            nc.vector.tensor_scalar_max(t[:], t[:], 1e-6)
                            nc.scalar.activation(
                                t[:], t[:], mybir.ActivationFunctionType.Ln
                            )
                            L.append(t)

                        sps = []
                        for st in range(2):
                            sp = psump.tile([P, Wimg], f32, tag=f"sp{st}",
                                            name=f"sp{st}")
                            for c2 in range(Wimg // mmchunk):
                                cols = slice(c2 * mmchunk, (c2 + 1) * mmchunk)
                                for c in range(3):
                                    nc.tensor.matmul(
                                        sp[:, cols],
                                        diags[3 * st + c][:],
                                        L[c][:, cols],
                                        start=(c == 0),
                                        stop=(c == 2),
                                    )
                            sps.append(sp)
                        Es = []
                        for st in range(2):
                            mvals = m_h if st == 0 else m_d
                            E = []
                            for d in range(3):
                                e = epool.tile([P, Wimg],
                                               f32r if USE_F32R else f32,
                                               tag=f"e{st}{d}", name=f"e{st}{d}")
                                nc.scalar.activation(
                                    e[:],
                                    sps[st][:],
                                    mybir.ActivationFunctionType.Exp,
                                    bias=bias_lnc[d][:],
                                    scale=float(mvals[d]),
                                )
                                E.append(e)
                            Es.append(E)
                        Gs = []
                        for st in range(2):
                            G = psump.tile([P, Wimg], f32, tag=f"G{st}",
                                           name=f"G{st}")
                            for c2 in range(Wimg // mmchunk):
                                cols = slice(c2 * mmchunk, (c2 + 1) * mmchunk)
                                for d in range(3):
                                    nc.tensor.matmul(
                                        G[:, cols],
                                        identr[:] if USE_F32R else diags[6][:],
                                        Es[st][d][:, cols],
                                        start=(d == 0),
                                        stop=(d == 2),
                                    )
                            Gs.append(G)
                        us = []
                        for st in range(2):
                            u = upool.tile([P, Wimg], f32, tag=f"u{st}",
                                           name=f"u{st}")
                            nc.scalar.activation(
                                u[:], Gs[st][:], mybir.ActivationFunctionType.Ln,
                                bias=bias_cal[:],
                            )
                            us.append(u)
                        for st in range(2):
                            si = img_i * 2 + st
                            fod = fodw[st][:, wcols]
                            nc.vector.tensor_scalar(
                                fod, us[st][:], -float(_INV_LN10), 0.0,
                                mybir.AluOpType.mult, mybir.AluOpType.max,
                            )
                            mk = maskp.tile([P, Wimg], f32)
                            nc.vector.tensor_scalar(
                                mk[:], fod, float(np.float32(_T_MASK)), None,
                                mybir.AluOpType.is_ge,
                            )
                            nc.sync.dma_start(masks_d[si][rows, :], mk[:])
                            for cb in range(4):
                                ccols = slice(bi * Wimg + cb * cbw,
                                              bi * Wimg + (cb + 1) * cbw)
                                nc.vector.scalar_tensor_tensor(
                                    scr_m[:, 0:cbw],
                                    fodw[st][:, ccols],
                                    float(np.float32(_T_FOD)),
                                    fodw[st][:, ccols],
                                    mybir.AluOpType.is_ge,
                                    mybir.AluOpType.mult,
                                    accum_out=bacc_sb[si][
                                        :, b * 4 + cb : b * 4 + cb + 1],
                                )

                    # wide-group histogram passes
                    for st in range(2):
                        si = img_i * 2 + st
                        for k in range(_NUM_BINS):
                            col = k * ng + g
                            if k in ACT_BINS:
                                nc.scalar.activation(
                                    scr_r[:], fodw[st][:],
                                    mybir.ActivationFunctionType.Relu,
                                    bias=bias_rk[k][:],
                                    accum_out=vacc_sb[si][:, col:col + 1],
                                )
                                nc.scalar.activation(
                                    scr_r[:], fodw[st][:],
                                    mybir.ActivationFunctionType.Sign,
                                    bias=bias_sk[k][:],
                                    accum_out=cacc_sb[si][:, col:col + 1],
                                )
                            else:
                                nc.vector.scalar_tensor_tensor(
                                    scr_m[:], fodw[st][:],
                                    edges[k],
                                    fodw[st][:],
                                    mybir.AluOpType.is_ge,
                                    mybir.AluOpType.mult,
                                    accum_out=vacc_sb[si][:, col:col + 1],
                                )

            for si in range(4):
                nc.sync.dma_start(vacc_d[si], vacc_sb[si][:])
                nc.sync.dma_start(cacc_d[si], cacc_sb[si][:])
                nc.sync.dma_start(bacc_d[si], bacc_sb[si][:])

    return dict(
        inputs=("x", "y", "diags", "identr"),
        outputs=("im_h", "im_d", "tm_h", "tm_d", "vacc", "cacc", "bacc"),
        nbands=nbands,
        ng=ng,
        wide=wide,
    )


# --------------------------------------------------------------------------
# host-side finishing
# --------------------------------------------------------------------------


def _finish_stats(vacc, cacc, bacc, nbands: int, ng: int, Wimg: int = W):
    """-> per stain-image: hist [20], blocks [4,4], avg (f64)."""
    Himg = nbands * P
    rows_per_block = Himg // 4
    grow = np.arange(nbands * P) // rows_per_block
    edges64 = np.float64([np.float32(k) * _BW32 for k in range(_NUM_BINS)])
    hists, blocks, avgs = [], [], []
    for si in range(4):
        V = vacc[si].reshape(P, _NUM_BINS, ng).sum(axis=(0, 2), dtype=np.float64)
        Sg = cacc[si].reshape(P, _NUM_BINS, ng).sum(axis=(0, 2), dtype=np.float64)
        Ntot = float(Himg) * float(Wimg)
        M = np.zeros(_NUM_BINS)
        for k in range(_NUM_BINS):
            if k in ACT_BINS:
                C_k = (Sg[k] + Ntot) / 2.0
                M[k] = V[k] + edges64[k] * C_k
            else:
                M[k] = V[k]
        bins = M.copy()
        bins[:-1] -= M[1:]
        hists.append(bins)
        per_row = bacc[si].reshape(P, nbands, 4).transpose(1, 0, 2).reshape(-1, 4)
        blk = np.zeros((4, 4))
        for r in range(4):
            blk[r] = per_row[grow == r].sum(axis=0, dtype=np.float64)
        blocks.append(blk)
        avgs.append(blk.sum())
    return hists, blocks, avgs


def _channel_loss(i_avg, i_blk, i_his, t_avg, t_blk, t_his, Bsz, HWsz):
    avg_t = (i_avg - t_avg) ** 2 / float(HWsz) ** 2
    his_t = np.sum((i_his / HWsz - t_his / HWsz) ** 2, axis=1) / Bsz
    blk_t = np.mean((i_blk / (HWsz / 16.0) - t_blk / (HWsz / 16.0)) ** 2)
    diff = i_avg - t_avg
    cond = (diff >= t_avg * -0.4) & (diff <= t_avg * 0.4)
    return np.sum(np.where(cond, his_t, avg_t + his_t)) + blk_t


_BUILT = {}
LAST_RESULTS = None


def _get_compiled():
    key = (H, W)
    if key not in _BUILT:
        nc = bacc.Bacc("TRN2", target_bir_lowering=False, debug=False)
        info = build_program(nc, H, W)
        nc.compile()
        _BUILT[key] = (nc, info)
    return _BUILT[key]


def kernel(inputs: np.ndarray, targets: np.ndarray):
    inputs = np.ascontiguousarray(np.asarray(inputs, dtype=np.float32))
    targets = np.ascontiguousarray(np.asarray(targets, dtype=np.float32))
    assert inputs.shape == (B, C, H, W)

    nc, info = _get_compiled()
    diags = _diag_consts()
    in_maps = [
        {"x": inputs[b], "y": targets[b], "diags": diags[:7],
         "identr": diags[7]} for b in range(B)
    ]
    trace = bool(int(os.environ.get("TRN_KERNEL_TRACE", "0")))
    res = bass_utils.run_bass_kernel_spmd(
        nc, in_maps, core_ids=list(range(B)), trace=trace
    )
    global LAST_RESULTS
    LAST_RESULTS = res
    results = res.results

    nbands, ng = H // P, info["ng"]
    im_h = np.stack([results[b]["im_h"] for b in range(B)])
    im_d = np.stack([results[b]["im_d"] for b in range(B)])
    tm_h = np.stack([results[b]["tm_h"] for b in range(B)])
    tm_d = np.stack([results[b]["tm_d"] for b in range(B)])

    ia = np.zeros((4, B)); ih = np.zeros((4, B, _NUM_BINS)); ib = np.zeros((4, B, 4, 4))
    for b in range(B):
        hists, blocks, avgs = _finish_stats(
            results[b]["vacc"], results[b]["cacc"], results[b]["bacc"],
            nbands, ng,
        )
        for si in range(4):
            ia[si, b] = avgs[si]
            ih[si, b] = hists[si]
            ib[si, b] = blocks[si]

    HWsz = H * W
    loss = _channel_loss(ia[0], ib[0], ih[0], ia[2], ib[2], ih[2], B, HWsz) + \
        _channel_loss(ia[1], ib[1], ih[1], ia[3], ib[3], ih[3], B, HWsz)

    return (np.float32(loss), im_h, tm_h, im_d, tm_d)
